# revision 31
# baseline (speedup 1.0000x reference)
"""Trainium2 Bass kernel for the Convpass-swin hypernet-fuse adapter module.

Data-parallel over batch: 32 samples -> 8 cores x 4 samples. Small weights are
replicated. The datapath is fp16 (PSUM accumulation stays fp32); qgelu is
computed as Silu(1.702*v)/1.702 with the 1/1.702 folded into the downstream
weights on the host, so each qgelu is a single ACT instruction.

Per-core dataflow (R = 4*28*28 = 3136 spatial rows, C=768, D=EMB=64):
  1. x is transposed on the HOST to xT [C, R] fp16; four 2-chunk DMAs
     alternate between the sync and scalar queues. Hypernet weights follow
     (even groups on sync up front, odd groups issued from inside the phase A
     loop). Constants are packed into 3 DMAs.
  2. Stacked matmul (K=C) computes meta1 and adapter-down together per
     half-sample chunk: PSUM [128, 392]. rows 0:64 -> ACT Relu(+b1) accum ->
     sum of h; rows 64:128 -> ACT Silu -> s1' = 1.702*qgelu(x_down) written
     into a zero-padded [128, 4, 30, 30] buffer (rows 64:128 at w,
     DVE-copied to rows 0:64 shifted one column so conv taps pair into K=128
     matmuls).
  3. prompt = (sum_h/784) @ w2.T (+ b2 + layer_emb) -> fused8 [128, 8]
     (j' halves of hyper_w stacked on the two K-halves).
  4. Hypernet: two matmuls (one per 512-wide weight tile) share a two-bank
     PSUM tile [8, 1024]; one cast moves both to SBUF. Casts alternate
     DVE/ACT per pair (even pairs -> stgA via DVE, odd -> stgB via ACT) so
     the cast rate is 2x one engine. 4 contiguous DMAs bounce stgA/stgB to
     DRAM as conv_w[b, (dw, di, dh, do)]/1.702.
  5. Conv: per-sample weight fetches (cwp on scalar, cws on sync) + bias
     adds are hoisted ahead of the conv loop. Per (sample, half): 3 paired
     K=128 + 3 single K=64 matmuls accumulate PSUM [64, 392]; ACT Silu ->
     yg_b [65, 784] fp16 (row 64 = ones).
  6. Up-projection interleaved per sample (conv0 conv1 up0 conv2 up1 ...):
     out[r, c] = yg_b.T @ [up_w.T/1.702; up_b] in 7 row-tiles of 112 into a
     per-sample [112, 5376] staging tile; ONE output DMA per sample; output
     is fp16, upcast on the host.
"""

import sys

sys.path.insert(0, "/opt/trn_rl_repo")

import numpy as np

import concourse.bass as bass
import concourse.tile as tile
from concourse import bacc, mybir
from concourse.bass_utils import run_bass_kernel_spmd

F32 = mybir.dt.float32
F16 = mybir.dt.float16
AF = mybir.ActivationFunctionType

B, H, W, C, D, EMB = 32, 28, 28, 768, 64, 64
NCORES = 8
BL = B // NCORES            # samples per core
R = BL * H * W              # 3136 rows per core
HP, WP = H + 2, W + 2       # padded 30x30
JTOT = D * D * 9            # 36864 hypernet outputs per sample
NHT = JTOT // 1024          # 36 hypernet weight tiles [128, 512]
HTG = 4                     # hypernet tiles per DMA group
NG = NHT // HTG             # 9 groups
NB = 392                    # half-sample chunk (14 rows of 28)
QS = 1.702                  # quick-gelu sigmoid scale

TRACE = False               # set True (e.g. from test.py) to capture a profile
LAST_EXEC_NS = None         # filled from the profile when TRACE is on

_cached = {}


def _build_program(with_bias=True):
    nc = bacc.Bacc("TRN2", target_bir_lowering=False, debug=False)

    xt = nc.declare_dram_parameter("xt", [C, R], F16, isOutput=False).ap()
    bigpk = nc.declare_dram_parameter("bigpk", [128, 960], F16, isOutput=False).ap()
    smallpk = nc.declare_dram_parameter("smallpk", [64, 67], F32, isOutput=False).ap()
    hwt = nc.declare_dram_parameter("hwt", [128, JTOT // 2], F16, isOutput=False).ap()
    hbp3 = nc.declare_dram_parameter("hbp3", [64, 192], F16, isOutput=False).ap()
    upw = nc.declare_dram_parameter("upw", [65, C], F16, isOutput=False).ap()
    out = nc.declare_dram_parameter("out", [R, C], F16, isOutput=True).ap()

    with tile.TileContext(nc) as tc, \
         tc.tile_pool(name="consts", bufs=1) as cpool, \
         tc.tile_pool(name="xin", bufs=1) as xinpool, \
         tc.tile_pool(name="hwp", bufs=NG) as hwpool, \
         tc.tile_pool(name="work", bufs=1) as wpool, \
         tc.tile_pool(name="cwtp", bufs=BL) as cwtpool, \
         tc.tile_pool(name="cwsp", bufs=BL) as cwspool, \
         tc.tile_pool(name="ygp", bufs=BL) as ygpool, \
         tc.tile_pool(name="outp", bufs=2) as outpool, \
         tc.tile_pool(name="dram", bufs=1, space="DRAM") as dpool:

        # ---------- x chunk 0 first, then packed constants ----------
        xtv = xt.rearrange("(t p) r -> p t r", p=128)
        xpieces = [(0, 1, nc.sync), (1, 2, nc.scalar), (2, 3, nc.sync),
                   (3, 4, nc.scalar), (4, 6, nc.sync), (6, 8, nc.scalar)]
        xc = [None] * 8
        for c0, c1, eng in xpieces:
            nch = c1 - c0
            xtile = xinpool.tile(
                [128, 6 * nch * NB], F16, tag=f"xc{c0}", name=f"xc{c0}"
            )
            eng.dma_start(
                out=xtile[:].rearrange("p (t r) -> p t r", t=6),
                in_=xtv[:, :, c0 * NB:c1 * NB],
            )
            for ci in range(c0, c1):
                xc[ci] = (xtile, nch, ci - c0)

        bigpk_sb = cpool.tile([128, 960], F16, tag="bigpk")
        nc.sync.dma_start(out=bigpk_sb[:], in_=bigpk)
        wstk_sb = bigpk_sb[:, 0:768]          # host pre-laid [p, (t, m)]
        hbp2_sb = bigpk_sb[:, 768:960]
        smallpk_sb = cpool.tile([64, 67], F32, tag="smallpk")
        nc.sync.dma_start(out=smallpk_sb[:], in_=smallpk)
        brelu_sb = smallpk_sb[:, 0:1]
        bsilu_sb = smallpk_sb[:, 1:2]
        fb_sb = smallpk_sb[:, 2:3]
        w2t_sb = smallpk_sb[:, 3:67]
        hbp3_sb = cpool.tile([64, 192], F16, tag="hbp3")
        nc.scalar.dma_start(out=hbp3_sb[:], in_=hbp3)

        s1pad = cpool.tile([128, BL * HP * WP], F16, tag="s1pad")
        nc.vector.memset(s1pad[:].bitcast(F32), 0.0)
        mha_sb = cpool.tile([64, 2 * BL], F32, tag="mha")
        mh_sb = cpool.tile([64, BL], F32, tag="mh")
        fused8 = cpool.tile([128, 2 * BL], F16, tag="fused8")
        cw_dram = dpool.tile([BL, JTOT], F16, tag="cw")

        s1v = s1pad[:].rearrange("p (b h w) -> p b h w", b=BL, h=HP, w=WP)

        # even hwt groups follow x on sync; odd groups are issued from inside
        # the phase A loop so they don't block early ACT work.
        hg = []
        for g in range(NG):
            ht = hwpool.tile([128, HTG * 512], F16, tag="hg", name=f"hg{g}")
            if g % 2 == 0:
                nc.sync.dma_start(
                    out=ht[:], in_=hwt[:, g * HTG * 512:(g + 1) * HTG * 512]
                )
            hg.append(ht)
        upw_sb = cpool.tile([65, C], F16, tag="upw")
        nc.sync.dma_start(out=upw_sb[:], in_=upw)

        # ---------- phase A: stacked meta1+down, prompt ----------
        with tc.tile_pool(name="stkps", bufs=2, space="PSUM") as stkpool, \
             tc.tile_pool(name="ppps", bufs=1, space="PSUM") as pppool:

            hsc = wpool.tile([64, NB], F16, tag="hsc")
            for ci in range(8):
                b, hc = divmod(ci, 2)
                ps = stkpool.tile([128, NB], F32, tag="stk", name="ps")
                xtile, nch, off = xc[ci]
                for kt in range(6):
                    x0 = (kt * nch + off) * NB
                    nc.tensor.matmul(
                        ps[:],
                        lhsT=wstk_sb[:, kt * 128:(kt + 1) * 128],
                        rhs=xtile[:, x0:x0 + NB],
                        start=(kt == 0),
                        stop=(kt == 5),
                    )
                nc.scalar.activation(
                    hsc[:], ps[0:64, :], AF.Relu,
                    bias=brelu_sb, accum_out=mha_sb[:, ci:ci + 1],
                )
                ps3 = ps[64:128, :].rearrange("p (h w) -> p h w", h=14, w=W)
                h0 = hc * 14 + 1
                nc.scalar.activation(
                    s1v[64:128, b, h0:h0 + 14, 0:W], ps3, AF.Silu,
                    bias=bsilu_sb, scale=QS,
                )
                nc.vector.tensor_copy(
                    out=s1v[0:64, b, h0:h0 + 14, 1:W + 1],
                    in_=s1v[64:128, b, h0:h0 + 14, 0:W],
                )
                if ci % 2 == 1:
                    nc.scalar.dma_start(
                        out=hg[ci][:],
                        in_=hwt[:, ci * HTG * 512:(ci + 1) * HTG * 512],
                    )

            mhv = mha_sb[:].rearrange("p (b h) -> p b h", b=BL)
            nc.vector.tensor_add(mh_sb[:], mhv[:, :, 0], mhv[:, :, 1])
            pp = pppool.tile([64, BL], F32, tag="pp")
            nc.tensor.matmul(
                pp[:], lhsT=w2t_sb, rhs=mh_sb[:], start=True, stop=True,
            )
            nc.vector.memset(fused8[:], 0.0)
            nc.scalar.activation(fused8[0:64, 0:BL], pp[:], AF.Identity, bias=fb_sb)
            nc.scalar.activation(
                fused8[64:128, BL:2 * BL], pp[:], AF.Identity, bias=fb_sb
            )

        # ---------- phase B: hypernet, conv, up-projection ----------
        # hwt rows 0:64 hold EMB for j' 0:18432, rows 64:128 for j' 18432:.
        # j' semantic layout (host permute): (dw, di, dh, do); conv fetch uses
        # partition = (dw, di) so [0:128] is the dw 0/1 pair and [128:192] dw=2.
        cwt4 = cw_dram[:].rearrange(
            "b (dwdi dhdo) -> b dwdi dhdo", dwdi=3 * D, dhdo=3 * D
        )
        # bounce view: jlo = m*2048 + si*1024 + f (si = cast parity)
        cwb = cw_dram[:].rearrange(
            "b (par m si f) -> b par m si f", par=2, m=NHT // 4, si=2, f=1024
        )

        def cast_copy(i, out_, in_):
            if i % 2 == 0:
                nc.vector.tensor_copy(out=out_, in_=in_)
            else:
                nc.scalar.copy(out_, in_)

        # hypernet: two matmuls share a two-bank PSUM tile [8, 1024]; one
        # cast per pair, alternating engines (even pair -> DVE -> stgA,
        # odd pair -> ACT -> stgB). Own PSUM scope so the banks free up for
        # the conv/up phase.
        with tc.tile_pool(name="cwps", bufs=3, space="PSUM") as cwpool:
            stgA = cpool.tile([8, JTOT // 4], F16, tag="stgA")
            stgB = cpool.tile([8, JTOT // 4], F16, tag="stgB")
            for cc in range(NHT // 2):
                cps2 = cwpool.tile([8, 1024], F32, tag="cw", name="cps2")
                for u in range(2):
                    ct = 2 * cc + u
                    nc.tensor.matmul(
                        cps2[:, u * 512:(u + 1) * 512],
                        lhsT=fused8[:],
                        rhs=hg[ct // HTG][:, (ct % HTG) * 512:(ct % HTG + 1) * 512],
                        start=True, stop=True,
                    )
                stg = stgA if cc % 2 == 0 else stgB
                f0 = (cc // 2) * 1024
                cast_copy(cc, stg[:, f0:f0 + 1024], cps2[:])
            for si, stg in enumerate((stgA, stgB)):
                sv = stg[:].rearrange("p (m f) -> p m f", f=1024)
                for par in range(2):
                    eng = nc.sync if par == 0 else nc.scalar
                    eng.dma_start(
                        out=cwb[0:BL, par, :, si, :],
                        in_=sv[par * BL:(par + 1) * BL],
                    )

            # prefetch every sample's conv weights (+ bias adds only when
            # hyper_b is nonzero) before the conv loop, split across queues
            cwps, cwss = [], []
            for b in range(BL):
                cwp = cwtpool.tile([128, 192], F16, tag="cwp", name=f"cwp{b}")
                nc.sync.dma_start(out=cwp[:], in_=cwt4[b, 0:128])
                cws = cwspool.tile([64, 192], F16, tag="cws", name=f"cws{b}")
                nc.scalar.dma_start(out=cws[:], in_=cwt4[b, 128:192])
                if with_bias:
                    nc.vector.tensor_add(cwp[:], cwp[:], hbp2_sb)
                    nc.vector.tensor_add(cws[:], cws[:], hbp3_sb[:])
                cwps.append(cwp)
                cwss.append(cws)

        with tc.tile_pool(name="cvps", bufs=2, space="PSUM") as cvpool, \
             tc.tile_pool(name="upps", bufs=3, space="PSUM") as uppool:

            yg = [None] * BL

            def conv(b):
                cwp, cws = cwps[b], cwss[b]
                ygb = ygpool.tile([65, H * W], F16, tag="yg", name=f"yg{b}")
                nc.vector.memset(ygb[64:65, :], 1.0)
                yg[b] = ygb
                for hc in range(2):
                    cvp = cvpool.tile([64, NB], F32, tag="cv")
                    cvp3 = cvp[:].rearrange("p (h w) -> p h w", h=14, w=W)
                    for dh in range(3):
                        r0 = hc * 14 + dh
                        nc.tensor.matmul(
                            cvp3,
                            lhsT=cwp[:, dh * 64:(dh + 1) * 64],
                            rhs=s1v[:, b, r0:r0 + 14, 0:W],
                            start=(dh == 0), stop=False,
                        )
                        nc.tensor.matmul(
                            cvp3,
                            lhsT=cws[:, dh * 64:(dh + 1) * 64],
                            rhs=s1v[0:64, b, r0:r0 + 14, 2:W + 2],
                            start=False, stop=(dh == 2),
                        )
                    nc.scalar.activation(
                        ygb[0:64, hc * NB:(hc + 1) * NB], cvp[:], AF.Silu, scale=QS,
                    )

            ov = out.rearrange("(b j p) c -> b p j c", b=BL, j=7, p=112)

            def up(b):
                osb = outpool.tile([112, 7 * C], F16, tag="osb", name="osb")
                for j in range(7):
                    # both halves share a two-bank PSUM tile; one strided cast
                    upp = uppool.tile([112, 1024], F32, tag="up", name="upp")
                    for ni, n0 in enumerate((0, 384)):
                        nc.tensor.matmul(
                            upp[:, ni * 512:ni * 512 + 384],
                            lhsT=yg[b][:, j * 112:(j + 1) * 112],
                            rhs=upw_sb[:, n0:n0 + 384],
                            start=True, stop=True,
                        )
                    uv = upp[:].rearrange("p (k f) -> p k f", k=2)
                    osv = osb[:, j * C:(j + 1) * C].rearrange(
                        "p (k f) -> p k f", k=2
                    )
                    cast_copy(j, osv[:, :, 0:384], uv[:, :, 0:384])
                nc.sync.dma_start(
                    out=ov[b], in_=osb[:].rearrange("p (j c) -> p j c", j=7)
                )

            for b in range(BL):
                conv(b)
                if b >= 1:
                    up(b - 1)
            up(BL - 1)

    nc.compile()
    return nc


def _prep_host(inputs):
    f32 = lambda a: np.ascontiguousarray(np.asarray(a, dtype=np.float32))
    x = f32(inputs["x"])
    meta_w1, meta_b1 = f32(inputs["meta_w1"]), f32(inputs["meta_b1"])
    meta_w2, meta_b2 = f32(inputs["meta_w2"]), f32(inputs["meta_b2"])
    layer_emb = f32(inputs["layer_emb"])
    hyper_w, hyper_b = f32(inputs["hyper_w"]), f32(inputs["hyper_b"])
    down_w, down_b = f32(inputs["down_w"]), f32(inputs["down_b"])
    up_w, up_b = f32(inputs["up_w"]), f32(inputs["up_b"])

    # stacked meta1+down weights, pre-laid as SBUF [p, (t, m)]
    wstk = np.concatenate([meta_w1, down_w], axis=0).T  # [C, 128]
    wstk2 = wstk.reshape(6, 128, 128).transpose(1, 0, 2).reshape(128, 768)

    # hyper_w [j, e], j = (do, di, kh, kw) -> [e, j'], j' = (kw, di, kh, do);
    # the 1/QS folds the Silu-form qgelu of the conv input.
    hw5 = (hyper_w / QS).reshape(D, D, 3, 3, EMB)     # do, di, kh, kw, e
    hwtp = hw5.transpose(4, 3, 1, 2, 0).reshape(EMB, JTOT)
    # stack the two j' halves on partition halves -> [128, JTOT/2]
    half = JTOT // 2
    hwt = np.concatenate(
        [hwtp[:, :half], hwtp[:, half:]], axis=0
    ).astype(np.float16)
    hb4 = (hyper_b / QS).reshape(D, D, 3, 3).transpose(3, 1, 2, 0)  # kw, di, kh, do
    hbp2 = hb4[0:2].reshape(128, 192)
    hbp3 = hb4[2].reshape(64, 192).astype(np.float16)

    bigpk = np.concatenate([wstk2, hbp2], axis=1).astype(np.float16)  # [128, 960]
    smallpk = np.concatenate(
        [meta_b1.reshape(64, 1), (QS * down_b).reshape(64, 1),
         (meta_b2 + layer_emb).reshape(64, 1), meta_w2.T / 784.0], axis=1,
    ).astype(np.float32)                                              # [64, 67]

    upw = np.concatenate(
        [up_w.T / QS, up_b.reshape(1, C)], axis=0
    ).astype(np.float16)  # [65, C]

    shared = dict(bigpk=np.ascontiguousarray(bigpk),
                  smallpk=np.ascontiguousarray(smallpk),
                  hwt=np.ascontiguousarray(hwt),
                  hbp3=np.ascontiguousarray(hbp3),
                  upw=np.ascontiguousarray(upw))
    in_maps = []
    for k in range(NCORES):
        m = dict(shared)
        m["xt"] = np.ascontiguousarray(
            x[k * BL:(k + 1) * BL].reshape(R, C).T.astype(np.float16)
        )
        in_maps.append(m)
    return in_maps


def kernel(**inputs) -> np.ndarray:
    with_bias = bool(np.any(np.asarray(inputs["hyper_b"])))
    key = f"nc{with_bias}"
    if key not in _cached:
        _cached[key] = _build_program(with_bias)
    nc = _cached[key]
    in_maps = _prep_host(inputs)
    res = run_bass_kernel_spmd(nc, in_maps, list(range(NCORES)), trace=TRACE)
    global LAST_EXEC_NS
    if TRACE and res.exec_time_ns is not None:
        LAST_EXEC_NS = res.exec_time_ns
        print(f"HW exec time: {res.exec_time_ns} ns")
    outs = [
        res.results[k]["out"].astype(np.float32).reshape(BL, H, W, C)
        for k in range(NCORES)
    ]
    return np.concatenate(outs, axis=0)


# revision 33
# speedup vs baseline: 1.1871x; 1.1871x over previous
"""Trainium2 Bass kernel for the Convpass-swin hypernet-fuse adapter module.

Data-parallel over batch: 32 samples -> 8 cores x 4 samples. Small weights are
replicated. The datapath is fp16 (PSUM accumulation stays fp32); qgelu is
computed as Silu(1.702*v)/1.702 with the 1/1.702 folded into the downstream
weights on the host, so each qgelu is a single ACT instruction.

Per-core dataflow (R = 4*28*28 = 3136 spatial rows, C=768, D=EMB=64):
  1. x is transposed on the HOST to xT [C, R] fp16; four 2-chunk DMAs
     alternate between the sync and scalar queues. Hypernet weights follow
     (even groups on sync up front, odd groups issued from inside the phase A
     loop). Constants are packed into 3 DMAs.
  2. Stacked matmul (K=C) computes meta1 and adapter-down together per
     half-sample chunk: PSUM [128, 392]. rows 0:64 -> ACT Relu(+b1) accum ->
     sum of h; rows 64:128 -> ACT Silu -> s1' = 1.702*qgelu(x_down) written
     into a zero-padded [128, 4, 30, 30] buffer (rows 64:128 at w,
     DVE-copied to rows 0:64 shifted one column so conv taps pair into K=128
     matmuls).
  3. prompt = (sum_h/784) @ w2.T (+ b2 + layer_emb) -> fused8 [128, 8]
     (j' halves of hyper_w stacked on the two K-halves).
  4. Hypernet: two matmuls (one per 512-wide weight tile) share a two-bank
     PSUM tile [8, 1024]; one cast moves both to SBUF. Casts alternate
     DVE/ACT per pair (even pairs -> stgA via DVE, odd -> stgB via ACT) so
     the cast rate is 2x one engine. 4 contiguous DMAs bounce stgA/stgB to
     DRAM as conv_w[b, (dw, di, dh, do)]/1.702.
  5. Conv: per-sample weight fetches (cwp on scalar, cws on sync) + bias
     adds are hoisted ahead of the conv loop. Per (sample, half): 3 paired
     K=128 + 3 single K=64 matmuls accumulate PSUM [64, 392]; ACT Silu ->
     yg_b [65, 784] fp16 (row 64 = ones).
  6. Up-projection interleaved per sample (conv0 conv1 up0 conv2 up1 ...):
     out[r, c] = yg_b.T @ [up_w.T/1.702; up_b] in 7 row-tiles of 112 into a
     per-sample [112, 5376] staging tile; ONE output DMA per sample; output
     is fp16, upcast on the host.
"""

import sys

sys.path.insert(0, "/opt/trn_rl_repo")

import numpy as np

import concourse.bass as bass
import concourse.tile as tile
from concourse import bacc, mybir
from concourse.bass_utils import run_bass_kernel_spmd

F32 = mybir.dt.float32
F16 = mybir.dt.float16
AF = mybir.ActivationFunctionType

B, H, W, C, D, EMB = 32, 28, 28, 768, 64, 64
NCORES = 8
BL = B // NCORES            # samples per core
R = BL * H * W              # 3136 rows per core
HP, WP = H + 2, W + 2       # padded 30x30
JTOT = D * D * 9            # 36864 hypernet outputs per sample
NHT = JTOT // 1024          # 36 hypernet weight tiles [128, 512]
HTG = 4                     # hypernet tiles per DMA group
NG = NHT // HTG             # 9 groups
NB = 392                    # half-sample chunk (14 rows of 28)
QS = 1.702                  # quick-gelu sigmoid scale

TRACE = False               # set True (e.g. from test.py) to capture a profile
LAST_EXEC_NS = None         # filled from the profile when TRACE is on

_cached = {}


def _build_program(with_bias=True):
    nc = bacc.Bacc("TRN2", target_bir_lowering=False, debug=False)

    xt = nc.declare_dram_parameter("xt", [C, R], F16, isOutput=False).ap()
    bigpk = nc.declare_dram_parameter("bigpk", [128, 960], F16, isOutput=False).ap()
    smallpk = nc.declare_dram_parameter("smallpk", [64, 67], F32, isOutput=False).ap()
    hwt = nc.declare_dram_parameter("hwt", [128, JTOT // 2], F16, isOutput=False).ap()
    hbp3 = nc.declare_dram_parameter("hbp3", [64, 192], F16, isOutput=False).ap()
    upw = nc.declare_dram_parameter("upw", [65, C], F16, isOutput=False).ap()
    out = nc.declare_dram_parameter("out", [R, C], F16, isOutput=True).ap()

    with tile.TileContext(nc) as tc, \
         tc.tile_pool(name="consts", bufs=1) as cpool, \
         tc.tile_pool(name="xin", bufs=1) as xinpool, \
         tc.tile_pool(name="hwp", bufs=NG) as hwpool, \
         tc.tile_pool(name="work", bufs=1) as wpool, \
         tc.tile_pool(name="cwtp", bufs=BL) as cwtpool, \
         tc.tile_pool(name="cwsp", bufs=BL) as cwspool, \
         tc.tile_pool(name="ygp", bufs=BL) as ygpool, \
         tc.tile_pool(name="outp", bufs=2) as outpool, \
         tc.tile_pool(name="dram", bufs=1, space="DRAM") as dpool:

        # ---------- x chunk 0 + packed constants first, then the rest ----------
        xtv = xt.rearrange("(t p) r -> p t r", p=128)
        xpieces = [(0, 1, nc.sync), (1, 2, nc.scalar), (2, 3, nc.sync),
                   (3, 4, nc.scalar), (4, 6, nc.sync), (6, 8, nc.scalar)]
        xc = [None] * 8

        def xissue(c0, c1, eng):
            nch = c1 - c0
            xtile = xinpool.tile(
                [128, 6 * nch * NB], F16, tag=f"xc{c0}", name=f"xc{c0}"
            )
            eng.dma_start(
                out=xtile[:].rearrange("p (t r) -> p t r", t=6),
                in_=xtv[:, :, c0 * NB:c1 * NB],
            )
            for ci in range(c0, c1):
                xc[ci] = (xtile, nch, ci - c0)

        xissue(*xpieces[0])
        xissue(*xpieces[1])
        bigpk_sb = cpool.tile([128, 960], F16, tag="bigpk")
        nc.sync.dma_start(out=bigpk_sb[:], in_=bigpk)
        wstk_sb = bigpk_sb[:, 0:768]          # host pre-laid [p, (t, m)]
        hbp2_sb = bigpk_sb[:, 768:960]
        smallpk_sb = cpool.tile([64, 67], F32, tag="smallpk")
        nc.sync.dma_start(out=smallpk_sb[:], in_=smallpk)
        brelu_sb = smallpk_sb[:, 0:1]
        bsilu_sb = smallpk_sb[:, 1:2]
        fb_sb = smallpk_sb[:, 2:3]
        w2t_sb = smallpk_sb[:, 3:67]
        hbp3_sb = cpool.tile([64, 192], F16, tag="hbp3")
        nc.scalar.dma_start(out=hbp3_sb[:], in_=hbp3)
        for pc in xpieces[2:]:
            xissue(*pc)

        s1pad = cpool.tile([128, BL * HP * WP], F16, tag="s1pad")
        nc.vector.memset(s1pad[:].bitcast(F32), 0.0)
        mha_sb = cpool.tile([64, 2 * BL], F32, tag="mha")
        mh_sb = cpool.tile([64, BL], F32, tag="mh")
        fused8 = cpool.tile([128, 2 * BL], F16, tag="fused8")
        cw_dram = dpool.tile([BL, JTOT], F16, tag="cw")

        s1v = s1pad[:].rearrange("p (b h w) -> p b h w", b=BL, h=HP, w=WP)

        # even hwt groups follow x on sync; odd groups are issued from inside
        # the phase A loop so they don't block early ACT work.
        hg = []
        for g in range(NG):
            ht = hwpool.tile([128, HTG * 512], F16, tag="hg", name=f"hg{g}")
            if g % 2 == 0:
                nc.sync.dma_start(
                    out=ht[:], in_=hwt[:, g * HTG * 512:(g + 1) * HTG * 512]
                )
            hg.append(ht)
        upw_sb = cpool.tile([65, C], F16, tag="upw")
        nc.sync.dma_start(out=upw_sb[:], in_=upw)

        # ---------- phase A: stacked meta1+down, prompt ----------
        with tc.tile_pool(name="stkps", bufs=2, space="PSUM") as stkpool, \
             tc.tile_pool(name="ppps", bufs=1, space="PSUM") as pppool:

            hsc = wpool.tile([64, NB], F16, tag="hsc")
            for ci in range(8):
                b, hc = divmod(ci, 2)
                ps = stkpool.tile([128, NB], F32, tag="stk", name="ps")
                xtile, nch, off = xc[ci]
                for kt in range(6):
                    x0 = (kt * nch + off) * NB
                    nc.tensor.matmul(
                        ps[:],
                        lhsT=wstk_sb[:, kt * 128:(kt + 1) * 128],
                        rhs=xtile[:, x0:x0 + NB],
                        start=(kt == 0),
                        stop=(kt == 5),
                    )
                nc.scalar.activation(
                    hsc[:], ps[0:64, :], AF.Relu,
                    bias=brelu_sb, accum_out=mha_sb[:, ci:ci + 1],
                )
                ps3 = ps[64:128, :].rearrange("p (h w) -> p h w", h=14, w=W)
                h0 = hc * 14 + 1
                nc.scalar.activation(
                    s1v[64:128, b, h0:h0 + 14, 0:W], ps3, AF.Silu,
                    bias=bsilu_sb, scale=QS,
                )
                nc.vector.tensor_copy(
                    out=s1v[0:64, b, h0:h0 + 14, 1:W + 1],
                    in_=s1v[64:128, b, h0:h0 + 14, 0:W],
                )
                if ci % 2 == 1:
                    nc.scalar.dma_start(
                        out=hg[ci][:],
                        in_=hwt[:, ci * HTG * 512:(ci + 1) * HTG * 512],
                    )

            mhv = mha_sb[:].rearrange("p (b h) -> p b h", b=BL)
            nc.vector.tensor_add(mh_sb[:], mhv[:, :, 0], mhv[:, :, 1])
            pp = pppool.tile([64, BL], F32, tag="pp")
            nc.tensor.matmul(
                pp[:], lhsT=w2t_sb, rhs=mh_sb[:], start=True, stop=True,
            )
            nc.vector.memset(fused8[:], 0.0)
            nc.scalar.activation(fused8[0:64, 0:BL], pp[:], AF.Identity, bias=fb_sb)
            nc.scalar.activation(
                fused8[64:128, BL:2 * BL], pp[:], AF.Identity, bias=fb_sb
            )

        # ---------- phase B: hypernet, conv, up-projection ----------
        # hwt rows 0:64 hold EMB for j' 0:18432, rows 64:128 for j' 18432:.
        # j' semantic layout (host permute): (dw, di, dh, do); conv fetch uses
        # partition = (dw, di) so [0:128] is the dw 0/1 pair and [128:192] dw=2.
        cwt4 = cw_dram[:].rearrange(
            "b (dwdi dhdo) -> b dwdi dhdo", dwdi=3 * D, dhdo=3 * D
        )
        # bounce view: jlo = m*2048 + si*1024 + f (si = cast parity)
        cwb = cw_dram[:].rearrange(
            "b (par m si f) -> b par m si f", par=2, m=NHT // 4, si=2, f=1024
        )

        def cast_copy(i, out_, in_):
            if i % 2 == 0:
                nc.vector.tensor_copy(out=out_, in_=in_)
            else:
                nc.scalar.copy(out_, in_)

        # hypernet: two matmuls share a two-bank PSUM tile [8, 1024]; one
        # cast per pair, alternating engines (even pair -> DVE -> stgA,
        # odd pair -> ACT -> stgB). Own PSUM scope so the banks free up for
        # the conv/up phase.
        with tc.tile_pool(name="cwps", bufs=3, space="PSUM") as cwpool:
            stgA = cpool.tile([8, JTOT // 4], F16, tag="stgA")
            stgB = cpool.tile([8, JTOT // 4], F16, tag="stgB")
            for cc in range(NHT // 2):
                cps2 = cwpool.tile([8, 1024], F32, tag="cw", name="cps2")
                for u in range(2):
                    ct = 2 * cc + u
                    nc.tensor.matmul(
                        cps2[:, u * 512:(u + 1) * 512],
                        lhsT=fused8[:],
                        rhs=hg[ct // HTG][:, (ct % HTG) * 512:(ct % HTG + 1) * 512],
                        start=True, stop=True,
                    )
                stg = stgA if cc % 2 == 0 else stgB
                f0 = (cc // 2) * 1024
                cast_copy(cc, stg[:, f0:f0 + 1024], cps2[:])
            for si, stg in enumerate((stgA, stgB)):
                sv = stg[:].rearrange("p (m f) -> p m f", f=1024)
                for par in range(2):
                    eng = nc.sync if par == 0 else nc.scalar
                    eng.dma_start(
                        out=cwb[0:BL, par, :, si, :],
                        in_=sv[par * BL:(par + 1) * BL],
                    )

            # prefetch every sample's conv weights (+ bias adds only when
            # hyper_b is nonzero) before the conv loop, split across queues
            cwps, cwss = [], []
            for b in range(BL):
                cwp = cwtpool.tile([128, 192], F16, tag="cwp", name=f"cwp{b}")
                nc.sync.dma_start(out=cwp[:], in_=cwt4[b, 0:128])
                cws = cwspool.tile([64, 192], F16, tag="cws", name=f"cws{b}")
                nc.scalar.dma_start(out=cws[:], in_=cwt4[b, 128:192])
                if with_bias:
                    nc.vector.tensor_add(cwp[:], cwp[:], hbp2_sb)
                    nc.vector.tensor_add(cws[:], cws[:], hbp3_sb[:])
                cwps.append(cwp)
                cwss.append(cws)

        with tc.tile_pool(name="cvps", bufs=2, space="PSUM") as cvpool, \
             tc.tile_pool(name="upps", bufs=3, space="PSUM") as uppool:

            yg = [None] * BL

            def conv(b):
                cwp, cws = cwps[b], cwss[b]
                ygb = ygpool.tile([65, H * W], F16, tag="yg", name=f"yg{b}")
                nc.vector.memset(ygb[64:65, :], 1.0)
                yg[b] = ygb
                for hc in range(2):
                    cvp = cvpool.tile([64, NB], F32, tag="cv")
                    cvp3 = cvp[:].rearrange("p (h w) -> p h w", h=14, w=W)
                    for dh in range(3):
                        r0 = hc * 14 + dh
                        nc.tensor.matmul(
                            cvp3,
                            lhsT=cwp[:, dh * 64:(dh + 1) * 64],
                            rhs=s1v[:, b, r0:r0 + 14, 0:W],
                            start=(dh == 0), stop=False,
                        )
                        nc.tensor.matmul(
                            cvp3,
                            lhsT=cws[:, dh * 64:(dh + 1) * 64],
                            rhs=s1v[0:64, b, r0:r0 + 14, 2:W + 2],
                            start=False, stop=(dh == 2),
                        )
                    nc.scalar.activation(
                        ygb[0:64, hc * NB:(hc + 1) * NB], cvp[:], AF.Silu, scale=QS,
                    )

            ov = out.rearrange("(b j p) c -> b p j c", b=BL, j=7, p=112)

            def up(b):
                osb = outpool.tile([112, 7 * C], F16, tag="osb", name="osb")
                for j in range(7):
                    # both halves share a two-bank PSUM tile; one strided cast
                    upp = uppool.tile([112, 1024], F32, tag="up", name="upp")
                    for ni, n0 in enumerate((0, 384)):
                        nc.tensor.matmul(
                            upp[:, ni * 512:ni * 512 + 384],
                            lhsT=yg[b][:, j * 112:(j + 1) * 112],
                            rhs=upw_sb[:, n0:n0 + 384],
                            start=True, stop=True,
                        )
                    uv = upp[:].rearrange("p (k f) -> p k f", k=2)
                    osv = osb[:, j * C:(j + 1) * C].rearrange(
                        "p (k f) -> p k f", k=2
                    )
                    cast_copy(j, osv[:, :, 0:384], uv[:, :, 0:384])
                nc.sync.dma_start(
                    out=ov[b], in_=osb[:].rearrange("p (j c) -> p j c", j=7)
                )

            for b in range(BL):
                conv(b)
                if b >= 1:
                    up(b - 1)
            up(BL - 1)

    nc.compile()
    return nc


def _prep_host(inputs):
    f32 = lambda a: np.ascontiguousarray(np.asarray(a, dtype=np.float32))
    x = f32(inputs["x"])
    meta_w1, meta_b1 = f32(inputs["meta_w1"]), f32(inputs["meta_b1"])
    meta_w2, meta_b2 = f32(inputs["meta_w2"]), f32(inputs["meta_b2"])
    layer_emb = f32(inputs["layer_emb"])
    hyper_w, hyper_b = f32(inputs["hyper_w"]), f32(inputs["hyper_b"])
    down_w, down_b = f32(inputs["down_w"]), f32(inputs["down_b"])
    up_w, up_b = f32(inputs["up_w"]), f32(inputs["up_b"])

    # stacked meta1+down weights, pre-laid as SBUF [p, (t, m)]
    wstk = np.concatenate([meta_w1, down_w], axis=0).T  # [C, 128]
    wstk2 = wstk.reshape(6, 128, 128).transpose(1, 0, 2).reshape(128, 768)

    # hyper_w [j, e], j = (do, di, kh, kw) -> [e, j'], j' = (kw, di, kh, do);
    # the 1/QS folds the Silu-form qgelu of the conv input.
    hw5 = (hyper_w / QS).reshape(D, D, 3, 3, EMB)     # do, di, kh, kw, e
    hwtp = hw5.transpose(4, 3, 1, 2, 0).reshape(EMB, JTOT)
    # stack the two j' halves on partition halves -> [128, JTOT/2]
    half = JTOT // 2
    hwt = np.concatenate(
        [hwtp[:, :half], hwtp[:, half:]], axis=0
    ).astype(np.float16)
    hb4 = (hyper_b / QS).reshape(D, D, 3, 3).transpose(3, 1, 2, 0)  # kw, di, kh, do
    hbp2 = hb4[0:2].reshape(128, 192)
    hbp3 = hb4[2].reshape(64, 192).astype(np.float16)

    bigpk = np.concatenate([wstk2, hbp2], axis=1).astype(np.float16)  # [128, 960]
    smallpk = np.concatenate(
        [meta_b1.reshape(64, 1), (QS * down_b).reshape(64, 1),
         (meta_b2 + layer_emb).reshape(64, 1), meta_w2.T / 784.0], axis=1,
    ).astype(np.float32)                                              # [64, 67]

    upw = np.concatenate(
        [up_w.T / QS, up_b.reshape(1, C)], axis=0
    ).astype(np.float16)  # [65, C]

    shared = dict(bigpk=np.ascontiguousarray(bigpk),
                  smallpk=np.ascontiguousarray(smallpk),
                  hwt=np.ascontiguousarray(hwt),
                  hbp3=np.ascontiguousarray(hbp3),
                  upw=np.ascontiguousarray(upw))
    in_maps = []
    for k in range(NCORES):
        m = dict(shared)
        m["xt"] = np.ascontiguousarray(
            x[k * BL:(k + 1) * BL].reshape(R, C).T.astype(np.float16)
        )
        in_maps.append(m)
    return in_maps


def kernel(**inputs) -> np.ndarray:
    with_bias = bool(np.any(np.asarray(inputs["hyper_b"])))
    key = f"nc{with_bias}"
    if key not in _cached:
        _cached[key] = _build_program(with_bias)
    nc = _cached[key]
    in_maps = _prep_host(inputs)
    res = run_bass_kernel_spmd(nc, in_maps, list(range(NCORES)), trace=TRACE)
    global LAST_EXEC_NS
    if TRACE and res.exec_time_ns is not None:
        LAST_EXEC_NS = res.exec_time_ns
        print(f"HW exec time: {res.exec_time_ns} ns")
    outs = [
        res.results[k]["out"].astype(np.float32).reshape(BL, H, W, C)
        for k in range(NCORES)
    ]
    return np.concatenate(outs, axis=0)
